# revision 30
# baseline (speedup 1.0000x reference)
"""Trainium2 Bass kernel for the Augmented Neural ODE problem.

Strategy (hardcoded for the known shapes):
  - Data-parallel: shard the batch (1024) across 8 NeuronCores, 128 samples
    each.  MLP weights are replicated to every core.
  - Feature-major layout on chip: activation tiles are (features on
    partitions, samples free).  Matmuls keep weights stationary so layer
    outputs chain into the next contraction with no transposes.
  - Matmul inputs bf16 (1 cycle/row on the PE); PSUM accumulation, RK state
    and stage combinations fp32.  (CPU-emulated rel err of this mix ~2.6e-4.)
  - Layer biases fold into PSUM as rank-2 matmuls (bias rows x indicator),
    so no separate bias pass exists anywhere.
  - Split-L1: stage input u_s = acc_s + c*k_{s-1} is never materialized.
    W1^T acc_s runs early off the critical path; W1^T kc_{s-1} lands the
    moment kc_{s-1} does (the dopri5 diagonal coefficient is folded into
    the k op itself, so the unscaled W1 is reused and all combination
    coefficients become dt-free ratios).  The same split carries the
    y-update across step boundaries, so every stage runs the identical
    short chain.
  - Each layer's PSUM is split across two banks so the Scalar engine can
    tanh the first half while the Tensor engine still writes the second
    (same-bank PE-write/ACT-read is fatal on TRN2), pipelining tanh with
    the downstream matmuls.
  - kc = c*(L3 + b3) runs as one fused Vector-engine op (per-partition
    bias add + scalar mult, PSUM -> bf16 SBUF); all dopri5 combination
    updates run eagerly on the Vector engine right after each kc, off the
    critical path.
  - Fully unrolled; state stays in SBUF; outputs DMA out once per interval.
"""

import numpy as np
import ml_dtypes

LATENT = 123
AUG = 5
TOTAL = 128          # LATENT + AUG
HID = 512
B = 1024
T = 8
SUBSTEPS = 6
NCORES = 8
S = B // NCORES      # samples per core
KC = HID // 128      # 4 chunks of 128 along the hidden dim
HALF = HID // 2

# dopri5 tableau (lower-triangular stage coefficients + 5th-order weights)
RK_A = [
    [0.2],
    [3.0 / 40.0, 9.0 / 40.0],
    [44.0 / 45.0, -56.0 / 15.0, 32.0 / 9.0],
    [19372.0 / 6561.0, -25360.0 / 2187.0, 64448.0 / 6561.0, -212.0 / 729.0],
    [9017.0 / 3168.0, -355.0 / 33.0, 46732.0 / 5247.0, 49.0 / 176.0,
     -5103.0 / 18656.0],
]
RK_B = [35.0 / 384.0, 0.0, 500.0 / 1113.0, 125.0 / 192.0, -2187.0 / 6784.0,
        11.0 / 84.0]

BF16 = ml_dtypes.bfloat16

# Exposed for the dev harness (test.py): last BassKernelResults, and build
# overrides for reduced-size bring-up runs.
LAST_RESULT = None
CONFIG = {"n_intervals": T - 1, "substeps": SUBSTEPS, "mm_dtype": "bfloat16"}


# Per-stage k scaling: kc_j = DSC[j-1]*dt*(raw_j + b3).  Stages 1..5 carry
# their consumer's diagonal coefficient A[j][j]; k6 carries the y-update
# weight b6.  Combination updates then use dt-free coefficient RATIOS.
DSC = [RK_A[i][i] for i in range(5)] + [RK_B[5]]


def _build_program(dts, n_intervals, substeps, mm_dtype_name="bfloat16",
                   repeat=1):
    """Build the Bass program. dts: per-interval substep sizes (floats).

    repeat > 1 re-runs the whole integration from the evolved state — used
    only by the dev harness to measure per-iteration HW time by wall-clock
    slope (dispatch overhead cancels in the difference).
    """
    import concourse.tile as tile
    from concourse import bacc, mybir

    fp32 = mybir.dt.float32
    mmdt = getattr(mybir.dt, mm_dtype_name)

    # Bacc (not plain Bass): its finalize() runs generate_event_semaphores,
    # which splits multi-sem waits down to the 1-wait-per-instruction HW limit.
    nc = bacc.Bacc(None, target_bir_lowering=False)

    # ---- DRAM parameters (per core) ----
    zT_d = nc.declare_dram_parameter("zT", [TOTAL, S], fp32, isOutput=False)
    w1_d = nc.declare_dram_parameter("W1m", [TOTAL, HID], mmdt, isOutput=False)
    w2_d = nc.declare_dram_parameter("W2m", [KC, 128, HID], mmdt, isOutput=False)
    w3_d = nc.declare_dram_parameter("W3m", [KC, 128, TOTAL], mmdt, isOutput=False)
    ind_d = nc.declare_dram_parameter("IND2", [2, HALF], mmdt, isOutput=False)
    ind3_d = nc.declare_dram_parameter("IND3", [3, 384], mmdt, isOutput=False)
    b1h_d = nc.declare_dram_parameter("b1h", [2, 2, 128], mmdt, isOutput=False)
    b2a_d = nc.declare_dram_parameter("b2a", [3, 128], mmdt, isOutput=False)
    b2b_d = nc.declare_dram_parameter("b2b", [1, 128], mmdt, isOutput=False)
    b3_d = nc.declare_dram_parameter("b3c", [TOTAL, 1], fp32, isOutput=False)
    ys_d = nc.declare_dram_parameter(
        "ys", [n_intervals, TOTAL, S], fp32, isOutput=True)

    Tanh = mybir.ActivationFunctionType.Tanh
    mult = mybir.AluOpType.mult
    add = mybir.AluOpType.add

    with tile.TileContext(nc) as tc:
        with (
            tc.tile_pool(name="weights", bufs=1) as wpool,
            tc.tile_pool(name="state", bufs=1) as spool,
            tc.tile_pool(name="work", bufs=3) as work,
            tc.tile_pool(name="psum1", bufs=2, space="PSUM") as ppool1,
            tc.tile_pool(name="psum2", bufs=1, space="PSUM") as ppool2,
            tc.tile_pool(name="psum3", bufs=2, space="PSUM") as ppool3,
        ):
            # ---- load weights / biases (resident) ----
            w1 = wpool.tile([128, HID], mmdt)          # lhsT chunks: w1[:, c*128:]
            nc.gpsimd.dma_start(out=w1, in_=w1_d[:, :])
            w2 = []
            for kk in range(KC):
                w2k = wpool.tile([128, HID], mmdt, tag=f"w2_{kk}", name=f"w2_{kk}")
                nc.gpsimd.dma_start(out=w2k, in_=w2_d[kk])
                w2.append(w2k)
            w3 = wpool.tile([128, KC * TOTAL], mmdt)   # w3[:, k*128:] = W3 rows k
            for kk in range(KC):
                nc.gpsimd.dma_start(out=w3[:, kk * TOTAL:(kk + 1) * TOTAL],
                                    in_=w3_d[kk])
            ind2 = wpool.tile([2, HALF], mmdt)
            nc.gpsimd.dma_start(out=ind2, in_=ind_d[:, :])
            ind3 = wpool.tile([3, 384], mmdt)
            nc.gpsimd.dma_start(out=ind3, in_=ind3_d[:, :])
            b1h = []
            for h in range(2):
                t1 = wpool.tile([2, 128], mmdt, tag=f"b1h{h}", name=f"b1h{h}")
                nc.gpsimd.dma_start(out=t1, in_=b1h_d[h])
                b1h.append(t1)
            b2h3 = [wpool.tile([3, 128], mmdt, tag="b2a", name="b2a"),
                    wpool.tile([1, 128], mmdt, tag="b2b", name="b2b")]
            nc.gpsimd.dma_start(out=b2h3[0], in_=b2a_d[:, :])
            nc.gpsimd.dma_start(out=b2h3[1], in_=b2b_d[:, :])
            b3c = wpool.tile([TOTAL, 1], fp32)
            nc.gpsimd.dma_start(out=b3c, in_=b3_d[:, :])

            # ---- state ----
            y = spool.tile([TOTAL, S], fp32)
            nc.gpsimd.dma_start(out=y, in_=zT_d[:, :])
            y_bf = spool.tile([TOTAL, S], mmdt)
            nc.vector.tensor_copy(y_bf, y)

            accY = spool.tile([TOTAL, S], fp32)

            def open_banks():
                """Allocate next L1 half-banks and land the bias matmuls
                (no data deps — they fill the PE while it waits for kc)."""
                p1 = []
                for h in range(2):
                    ph = ppool1.tile([128, HALF], fp32, tag=f"p1{h}",
                                     name=f"p1{h}")
                    nc.tensor.matmul(ph, b1h[h], ind2, start=True, stop=False)
                    p1.append(ph)
                return p1

            def acc_mms(p1, rhs_bf, close=False):
                """+= W1^T rhs into already-opened half-banks."""
                for h in range(2):
                    for cc in range(2):
                        c = 2 * h + cc
                        nc.tensor.matmul(p1[h][:, cc * 128:(cc + 1) * 128],
                                         w1[:, c * 128:(c + 1) * 128], rhs_bf,
                                         start=False, stop=close and cc == 1)

            def acc_part(rhs_bf, close=False):
                p1 = open_banks()
                acc_mms(p1, rhs_bf, close)
                return p1

            def k_part(p1, k_bf):
                """Close the L1 half-banks: += W1^T kc_prev (kc pre-scaled)."""
                for h in range(2):
                    for cc in range(2):
                        c = 2 * h + cc
                        nc.tensor.matmul(p1[h][:, cc * 128:(cc + 1) * 128],
                                         w1[:, c * 128:(c + 1) * 128], k_bf,
                                         start=False, stop=cc == 1)

            def rest_of_eval(p1, tag, kscale):
                """tanh -> L2 -> tanh -> L3 -> k, half-bank pipelined."""
                h1 = work.tile([128, HID], mmdt, tag="h1", name="h1")
                for h in range(2):
                    nc.scalar.activation(h1[:, h * HALF:(h + 1) * HALF],
                                         p1[h], Tanh)

                # p2 split 3+1: p2a = m0..2 completes early for a long tanh2a
                # that overlaps the p2b tail; tanh2b is then short.
                p2a = ppool2.tile([128, 3 * 128], fp32, tag="p2a", name="p2a")
                p2b = ppool2.tile([128, 128], fp32, tag="p2b", name="p2b")
                nc.tensor.matmul(p2a, b2h3[0], ind3, start=True, stop=False)
                nc.tensor.matmul(p2b, b2h3[1], ind3[0:1, 0:128],
                                 start=True, stop=False)
                # p2a's contributions first (within each h1-half gate), so it
                # completes as early as possible
                for m, c in [(m, c) for m in (0, 1, 2) for c in (0, 1)] + \
                            [(3, 0), (3, 1)] + \
                            [(m, c) for m in (0, 1, 2) for c in (2, 3)] + \
                            [(3, 2), (3, 3)]:
                    if m < 3:
                        out_ap = p2a[:, m * 128:(m + 1) * 128]
                    else:
                        out_ap = p2b
                    nc.tensor.matmul(out_ap,
                                     w2[c][:, m * 128:(m + 1) * 128],
                                     h1[:, c * 128:(c + 1) * 128],
                                     start=False, stop=c == 3)
                h2 = work.tile([128, HID], mmdt, tag="h2", name="h2")
                nc.scalar.activation(h2[:, 0:384], p2a, Tanh)
                nc.scalar.activation(h2[:, 384:512], p2b, Tanh)

                p3 = ppool3.tile([TOTAL, S], fp32, tag="p3", name="p3")
                for c in range(KC):
                    nc.tensor.matmul(p3, w3[:, c * TOTAL:(c + 1) * TOTAL],
                                     h2[:, c * 128:(c + 1) * 128],
                                     start=(c == 0), stop=(c == KC - 1))
                # kc = kscale*(p3 + b3), PSUM -> bf16 SBUF on the Vector
                # engine; the scale folds the dopri5 diagonal coefficient so
                # the stage-input matmul reuses the unscaled W1.
                k = work.tile([TOTAL, S], mmdt, tag=f"k_{tag}", name=f"k_{tag}")
                nc.vector.tensor_scalar(k, p3, b3c, kscale, op0=add, op1=mult)
                return k

            # acc tiles for stages 4..6; acc{t} accumulates
            # y + sum_{j<=t-2} dt*A[t-2][j-1]*k_j in fp32, with the last
            # update emitting the bf16 copy for the matmul.
            accf = {t: spool.tile([TOTAL, S], fp32, tag=f"accf_{t}",
                                  name=f"accf_{t}") for t in (4, 5, 6)}

            def accbf_tile(t):
                return work.tile([TOTAL, S], mmdt, tag=f"accbf_{t}",
                                 name=f"accbf_{t}")

            # ---- integration ----
            # p1 banks for the very first evaluation: u = z
            p1_next = acc_part(y_bf, close=True)
            pending_k = None

            for rep_it in range(repeat * n_intervals):
                it = rep_it % n_intervals
                dt = float(dts[it])
                for st in range(substeps):
                    accbf = {}
                    k_prev = None
                    for s in range(1, 7):           # stages; kc_s produced
                        p1 = p1_next
                        # next banks' bias matmuls fill the kc wait
                        p1_next = open_banks()
                        if s > 1:
                            k_part(p1, k_prev)
                        elif pending_k is not None:
                            k_part(p1, pending_k)
                        # W1^T acc into next banks (off critical path)
                        if s < 6:
                            rhs = y_bf if s == 1 else accbf[s + 1]
                            acc_mms(p1_next, rhs)
                        else:
                            # next step's stage 1: u = y_new = accY + kc6
                            acc_mms(p1_next, accbf[1])

                        k = rest_of_eval(p1, f"s{s}", dt * DSC[s - 1])

                        # eager combination updates on this kc (off chain);
                        # coefficients are dt-free ratios vs the k scale
                        for t_ in range(s + 2, 7):
                            cij = RK_A[t_ - 2][s - 1] / DSC[s - 1]
                            is_final = t_ == s + 2
                            in1 = y if s == 1 else accf[t_]
                            if is_final:
                                ob = accbf_tile(t_)
                                nc.vector.scalar_tensor_tensor(
                                    ob, k, cij, in1, op0=mult, op1=add)
                                accbf[t_] = ob
                            else:
                                nc.vector.scalar_tensor_tensor(
                                    accf[t_], k, cij, in1, op0=mult, op1=add)
                        # y-accumulator (RK_B); b2 == 0
                        if s == 1:
                            nc.vector.scalar_tensor_tensor(
                                accY, k, RK_B[0] / DSC[0], y, op0=mult, op1=add)
                        elif s in (3, 4):
                            nc.vector.scalar_tensor_tensor(
                                accY, k, RK_B[s - 1] / DSC[s - 1], accY,
                                op0=mult, op1=add)
                        elif s == 5:
                            nc.vector.scalar_tensor_tensor(
                                accY, k, RK_B[4] / DSC[4], accY,
                                op0=mult, op1=add)
                            # bf16 copy feeds next step's stage-1 acc matmuls
                            ob = accbf_tile(1)
                            nc.vector.tensor_copy(ob, accY)
                            accbf[1] = ob
                        elif s == 6:
                            # y <- accY + 1.0*kc6 (state update, fp32)
                            nc.vector.scalar_tensor_tensor(
                                y, k, 1.0, accY, op0=mult, op1=add)
                            nc.vector.tensor_copy(y_bf, y)
                        k_prev = k
                    # kc6 feeds next step's stage-1 banks
                    pending_k = k_prev
                # store interval output
                nc.sync.dma_start(out=ys_d[it], in_=y)

    nc.compile()
    return nc


def _prep_in_maps(z0, W1, b1, W2, b2, W3, b3):
    """Host-side per-core input prep (weights replicated, batch sharded)."""
    mmnp = BF16 if CONFIG["mm_dtype"] == "bfloat16" else np.float32
    W1m = W1.astype(mmnp)                                    # (128, 512)
    W2m = W2.reshape(KC, 128, HID).astype(mmnp)              # row chunks
    W3m = W3.reshape(KC, 128, TOTAL).astype(mmnp)
    IND2 = np.zeros((2, HALF), np.float32)
    for cc in range(2):
        IND2[cc, cc * 128:(cc + 1) * 128] = 1.0
    IND2 = IND2.astype(mmnp)
    IND3 = np.zeros((3, 384), np.float32)
    for cc in range(3):
        IND3[cc, cc * 128:(cc + 1) * 128] = 1.0
    IND3 = IND3.astype(mmnp)
    b1hh = b1.reshape(2, 2, 128).astype(mmnp)
    b2r = b2.reshape(4, 128).astype(mmnp)
    b2a = np.ascontiguousarray(b2r[0:3])
    b2b = np.ascontiguousarray(b2r[3:4])
    b3c = b3.reshape(TOTAL, 1).astype(np.float32)

    zfull = np.concatenate([z0, np.zeros((B, AUG), np.float32)], axis=1)

    in_maps = []
    for c in range(NCORES):
        zT = np.ascontiguousarray(zfull[c * S:(c + 1) * S].T)  # (TOTAL, S)
        in_maps.append(dict(zT=zT, W1m=W1m, W2m=W2m, W3m=W3m,
                            IND2=IND2, IND3=IND3, b1h=b1hh, b2a=b2a, b2b=b2b,
                            b3c=b3c))
    return in_maps


def kernel(**inputs):
    z0 = np.asarray(inputs["z0"], dtype=np.float32)
    t = np.asarray(inputs["t"], dtype=np.float32)
    W1 = np.asarray(inputs["W1"], dtype=np.float32)
    b1 = np.asarray(inputs["b1"], dtype=np.float32)
    W2 = np.asarray(inputs["W2"], dtype=np.float32)
    b2 = np.asarray(inputs["b2"], dtype=np.float32)
    W3 = np.asarray(inputs["W3"], dtype=np.float32)
    b3 = np.asarray(inputs["b3"], dtype=np.float32)

    from concourse.bass_utils import run_bass_kernel_spmd

    ts_sorted = np.sort(t[0])
    n_intervals = CONFIG["n_intervals"]
    substeps = CONFIG["substeps"]
    dts = (ts_sorted[1:] - ts_sorted[:-1]).astype(np.float32) / np.float32(substeps)

    nc = _build_program(dts, n_intervals, substeps, CONFIG["mm_dtype"])
    in_maps = _prep_in_maps(z0, W1, b1, W2, b2, W3, b3)

    global LAST_RESULT
    LAST_RESULT = run_bass_kernel_spmd(nc, in_maps, list(range(NCORES)))
    res = LAST_RESULT.results

    out = np.empty((B, n_intervals + 1, LATENT), dtype=np.float32)
    out[:, 0, :] = z0
    for c in range(NCORES):
        ys = np.asarray(res[c]["ys"])          # (n_intervals, TOTAL, S)
        out[c * S:(c + 1) * S, 1:, :] = ys.transpose(2, 0, 1)[:, :, :LATENT]
    return out
